# revision 42
# baseline (speedup 1.0000x reference)
"""DeepseekV3 MLA attention forward on 8 Trainium2 NeuronCores.

Sharding: core c -> batch c//4, head group c%4 (4 of 16 heads).

v2: the low-rank down-projections (stage A) are token-sharded across the
4 cores of each batch group instead of replicated. Each core computes the
down-projection + RMSNorm + k-rope for its own 128-token slice of token
chunks 1..3 (chunk 0 is computed in full by every core, which is cheaper
than a fourth gather), then three AllGathers (replica groups
[[0,1,2,3],[4,5,6,7]]) distribute the normalized latents. The gathers run
on the collective cores while the PE works through chunk 0, so they are
almost fully hidden.

Stage A is feature-major (weights stationary, tokens moving): outputs land
directly in the [feature, token] layout stages B/C consume, eliminating all
DMA transposes. Per-token RMS norms are computed with an all-ones stationary
matmul (partition reduction, replicated across partitions) so scaling is a
plain elementwise multiply. The remaining transposes (q-rope heads, attn
output) run on the PE via identity-matmul transpose.

B (up-projections), C (attention), D (wo projection) are interleaved per
512-token chunk so the output projection and DMA overlap attention for the
next chunk. Matmuls in bf16 with f32 PSUM accumulation; exp without max
subtraction; multiplicative causal mask tiles; ones-augmented V gives the
softmax denominator for free. wo partial products are returned in bf16 and
summed on host.
"""

import math

import numpy as np
import ml_dtypes

import concourse.bass as bass
import concourse.tile as tile
import concourse.mybir as mybir
from concourse import bacc
from concourse import masks as bmasks
from concourse.bass_utils import run_bass_kernel_spmd

BF16 = mybir.dt.bfloat16
F32 = mybir.dt.float32
AF = mybir.ActivationFunctionType

# ---- model config (hardcoded to match the problem spec) ----
HIDDEN = 2048
N_HEADS = 16
Q_LORA = 1536
KV_LORA = 512
NOPE = 128
ROPE = 64
VHD = 128
QHD = NOPE + ROPE  # 192
BASE = 10000.0
SCALE = 40.0
ORIG_MAX = 4096
BETA_FAST = 32
BETA_SLOW = 1
EPS = 1e-6
B = 2
S = 2048

N_CORES = 8
HL = 4           # heads per core
P = 128
KH = HIDDEN // P     # 16
KQ = Q_LORA // P     # 12
KKV = KV_LORA // P   # 4
NFT = KQ + KKV + 1   # 17 down-proj feature tiles (12 q, 4 kv, 1 rope)
NC_CHUNK = 4         # (legacy) uniform chunk count
CW = 512             # max chunk width / psum tile width
# token chunks: (start, width). A smaller tail chunk shortens the serial
# dependency chain behind the last all-gather.
CHUNKS = [(0, 512), (512, 512), (1024, 512), (1536, 384), (1920, 128)]
# per-core gather slice widths for chunks 1.. (width // 4)
SLICES = [(512 + sum(w for _, w in CHUNKS[1:1 + i]) // 4, w // 4)
          for i, (_, w) in enumerate(CHUNKS[1:])]
# hT_own column bases for the gather slices
_sl_bases = []
_acc = 512
for _, w in CHUNKS[1:]:
    _sl_bases.append((_acc, w // 4))
    _acc += w // 4

_m = 0.1 * math.log(SCALE) + 1.0
SOFT_SCALE = (QHD ** -0.5) * _m * _m

REPLICA_GROUPS = [[0, 1, 2, 3], [4, 5, 6, 7]]


def _yarn_cos_sin(seq_len):
    dim = ROPE
    ar = np.arange(0, dim, 2, dtype=np.float32)
    freq_extra = 1.0 / BASE ** (ar / dim)
    freq_inter = 1.0 / (SCALE * BASE ** (ar / dim))
    low = math.floor(dim * math.log(ORIG_MAX / (BETA_FAST * 2 * math.pi)) / (2 * math.log(BASE)))
    high = math.ceil(dim * math.log(ORIG_MAX / (BETA_SLOW * 2 * math.pi)) / (2 * math.log(BASE)))
    low, high = max(low, 0), min(high, dim - 1)
    denom = (high - low) if high != low else 0.001
    ramp = np.clip((np.arange(dim // 2, dtype=np.float32) - low) / denom, 0.0, 1.0)
    inv_freq_mask = 1.0 - ramp
    inv_freq = freq_inter * (1.0 - inv_freq_mask) + freq_extra * inv_freq_mask
    t = np.arange(seq_len, dtype=np.float32)
    freqs = np.outer(t, inv_freq)
    emb = np.concatenate([freqs, freqs], axis=-1)
    # mscale ratio is 1.0 for this config
    return np.cos(emb).astype(np.float32), np.sin(emb).astype(np.float32)


_PERM64 = np.concatenate([np.arange(0, 64, 2), np.arange(1, 64, 2)])


def _bf16(x):
    return np.ascontiguousarray(x.astype(ml_dtypes.bfloat16))


def _emit_stage_a(nc, tc, pp, gin, qnT0, cnT0, krotT, cosA_sb, sinA_sb,
                  ones_sb, eps_sb, hT_own, wqaT, wkvaT, cosA, sinA):
    """Down-projections, RMS norms, k-rope for this core's tokens.

    7 uniform 128-token stages: own slices of chunks 1..3 (staged to DRAM
    and all-gathered), then the 4 quarters of chunk 0 (kept local)."""
    stg_tiles = []
    with (
        tc.tile_pool(name="wA", bufs=1) as wA,
        tc.tile_pool(name="htp", bufs=3) as htp,
        tc.tile_pool(name="stgp", bufs=2) as stgp,
        tc.tile_pool(name="sqp", bufs=18) as sqp,
        tc.tile_pool(name="scrA", bufs=6) as scrA,
        tc.tile_pool(name="psq", bufs=2, space="PSUM") as psq,
        tc.tile_pool(name="pskv", bufs=1, space="PSUM") as pskv,
        tc.tile_pool(name="psnr", bufs=1, space="PSUM") as psnr,
    ):
        wqa_sb = wA.tile([P, KH, Q_LORA], BF16, tag="wqa")
        wkva_sb = wA.tile([P, KH, KV_LORA + ROPE], BF16, tag="wkva")

        # (hT_own column base, width, gather slice index or None, quarter)
        stages = [(b, w, i + 1, None) for i, (b, w) in enumerate(_sl_bases)]
        stages += [(m * P, P, None, m) for m in range(4)]
        nsl = len(_sl_bases)
        # prefetch the gather slices' tokens, then the weights (single
        # batched DMAs: per-transfer overhead dominates small copies)
        ht_pre = {}
        for i, (colbase, w, _, _) in enumerate(stages[:nsl]):
            ht_pre[colbase] = htp.tile([P, KH, P], BF16, tag="ht",
                                       name=f"htp{colbase}")
            nc.sync.dma_start(
                ht_pre[colbase][:, :, 0:w],
                hT_own[:, colbase:colbase + w].rearrange("(k p) t -> p k t", p=P))
            if i == 0:
                nc.sync.dma_start(
                    wkva_sb[:], wkvaT[:].rearrange("(k p) f -> p k f", p=P))
        nc.vector.tensor_copy(wqa_sb[0:1, 0:1, 0:1], wkva_sb[0:1, 0:1, 0:1])
        nc.vector.tensor_copy(wqa_sb[0:1, 0:1, 1:2], ht_pre[_sl_bases[2][0]][0:1, 0:1, 0:1])
        for wg in range(3):
            nc.sync.dma_start(
                wqa_sb[:, :, wg * CW:(wg + 1) * CW],
                wqaT[:, wg * CW:(wg + 1) * CW].rearrange(
                    "(k p) f -> p k f", p=P))
        nc.sync.dma_start(cosA_sb[:], cosA[:])
        nc.sync.dma_start(sinA_sb[:], sinA[:])
        for colbase, wd, gslice, quarter in stages:
            if colbase in ht_pre:
                ht = ht_pre[colbase]
            else:
                ht = htp.tile([P, KH, P], BF16, tag="ht",
                              name=f"ht{colbase}")
                nc.sync.dma_start(
                    ht[:, :, 0:wd], hT_own[:, colbase:colbase + wd].rearrange(
                        "(k p) t -> p k t", p=P))
            ps_q = psq.tile([P, KQ, P], F32, tag="psq")
            ps_kv = pskv.tile([P, KKV, P], F32, tag="kvps")
            ps_nr = psnr.tile([P, 3, P], F32, tag="nrps")
            for f2 in range(KKV):
                for k in range(KH):
                    nc.tensor.matmul(
                        ps_kv[:, f2, 0:wd], wkva_sb[:, k, f2 * P:(f2 + 1) * P],
                        ht[:, k, 0:wd], start=(k == 0), stop=(k == KH - 1))
            for k in range(KH):
                nc.tensor.matmul(
                    ps_nr[0:ROPE, 2, 0:wd],
                    wkva_sb[:, k, KV_LORA:KV_LORA + ROPE], ht[:, k, 0:wd],
                    start=(k == 0), stop=(k == KH - 1))
            # kv norm drains early (fills the PE idle while wqa streams in)
            sg = scrA.tile([P, 2, P], F32, tag="sg")
            for f2 in range(KKV):
                sq = sqp.tile([P, P], BF16, tag="sq")
                nc.scalar.activation(sq[:, 0:wd], ps_kv[:, f2, 0:wd], AF.Square)
                nc.tensor.matmul(ps_nr[:, 1, 0:wd], ones_sb[:],
                                 sq[:, 0:wd],
                                 start=(f2 == 0), stop=(f2 == KKV - 1))
            nc.scalar.activation(sg[:, 1, 0:wd], ps_nr[:, 1, 0:wd],
                                 AF.Sqrt, scale=1.0 / KV_LORA, bias=eps_sb[:])
            rkv = scrA.tile([P, P], F32, tag="rkv")
            nc.vector.reciprocal(rkv[:, 0:wd], sg[:, 1, 0:wd])

            for ft in range(KQ):
                for k in range(KH):
                    nc.tensor.matmul(
                        ps_q[:, ft, 0:wd], wqa_sb[:, k, ft * P:(ft + 1) * P],
                        ht[:, k, 0:wd], start=(k == 0), stop=(k == KH - 1))
            for ft in range(KQ):
                sq = sqp.tile([P, P], BF16, tag="sq")
                nc.scalar.activation(sq[:, 0:wd], ps_q[:, ft, 0:wd], AF.Square)
                nc.tensor.matmul(ps_nr[:, 0, 0:wd], ones_sb[:],
                                 sq[:, 0:wd],
                                 start=(ft == 0), stop=(ft == KQ - 1))
            nc.scalar.activation(sg[:, 0, 0:wd], ps_nr[:, 0, 0:wd],
                                 AF.Sqrt, scale=1.0 / Q_LORA, bias=eps_sb[:])
            rq = scrA.tile([P, P], F32, tag="rq")
            nc.vector.reciprocal(rq[:, 0:wd], sg[:, 0, 0:wd])

            if gslice is not None:
                stg = stgp.tile([P, NFT, P], BF16, tag="stg")
                stg_tiles.append(stg)
                dq = [stg[:, ft, 0:wd] for ft in range(KQ)]
                dc = [stg[:, KQ + f2, 0:wd] for f2 in range(KKV)]
                dkr = stg[0:ROPE, KQ + KKV, 0:wd]
                nc.vector.memset(stg[ROPE:P, KQ + KKV, 0:wd], 0.0)
            else:
                m0 = quarter * P
                dq = [qnT0[:, ft, m0:m0 + P] for ft in range(KQ)]
                dc = [cnT0[:, f2, m0:m0 + P] for f2 in range(KKV)]
                dkr = krotT[:, m0:m0 + P]
            for ft in range(KQ):
                nc.vector.tensor_mul(dq[ft], ps_q[:, ft, 0:wd], rq[:, 0:wd])
            for f2 in range(KKV):
                nc.vector.tensor_mul(dc[f2], ps_kv[:, f2, 0:wd], rkv[:, 0:wd])
            # decoupled rope on the shared key (feature-major, true scale)
            ca = cosA_sb[:, colbase:colbase + wd]
            sa = sinA_sb[:, colbase:colbase + wd]
            tmp = scrA.tile([ROPE, P], BF16, tag="tmpr")
            nc.vector.tensor_mul(dkr, ps_nr[0:ROPE, 2, 0:wd], ca)
            nc.vector.tensor_mul(tmp[0:32, 0:wd], ps_nr[32:ROPE, 2, 0:wd],
                                 sa[0:32])
            nc.vector.tensor_mul(tmp[32:ROPE, 0:wd], ps_nr[0:32, 2, 0:wd],
                                 sa[32:ROPE])
            nc.vector.tensor_add(dkr, dkr, tmp[0:ROPE, 0:wd])

            if gslice is not None:
                nc.sync.dma_start(
                    gin[gslice - 1][:].rearrange("f p t -> p f t"),
                    stg[:, :, 0:wd])
                nc.gpsimd.collective_compute(
                    "AllGather",
                    mybir.AluOpType.bypass,
                    replica_groups=REPLICA_GROUPS,
                    ins=[gin[gslice - 1].opt()],
                    outs=[gout_tiles[gslice - 1].opt()],
                )
    return stg_tiles


def _build_nc():
    nc = bacc.Bacc("TRN2", target_bir_lowering=False, debug=False,
                   num_devices=N_CORES)

    hT_own = nc.declare_dram_parameter("hT_own", [HIDDEN, 896], BF16, isOutput=False)
    wqaT = nc.declare_dram_parameter("wqaT", [HIDDEN, Q_LORA], BF16, isOutput=False)
    wkvaT = nc.declare_dram_parameter("wkvaT", [HIDDEN, KV_LORA + ROPE], BF16, isOutput=False)
    wqbT = nc.declare_dram_parameter("wqbT", [Q_LORA, HL * QHD], BF16, isOutput=False)
    wkvbT = nc.declare_dram_parameter("wkvbT", [KV_LORA, HL * (NOPE + VHD)], BF16, isOutput=False)
    woT = nc.declare_dram_parameter("woT", [HL * VHD, HIDDEN], BF16, isOutput=False)
    cosA = nc.declare_dram_parameter("cosA", [ROPE, 896], BF16, isOutput=False)
    sinA = nc.declare_dram_parameter("sinA", [ROPE, 896], BF16, isOutput=False)
    cos_tm = nc.declare_dram_parameter("cos_tm", [S, ROPE], BF16, isOutput=False)
    sin_tm_s = nc.declare_dram_parameter("sin_tm_s", [S, ROPE], BF16, isOutput=False)
    masks = nc.declare_dram_parameter("masks", [P, 896], BF16, isOutput=False)
    outT = nc.declare_dram_parameter("outT", [HIDDEN, S], BF16, isOutput=True)

    TT = S // P

    global gout_tiles
    with tile.TileContext(nc) as tc:
        with (
            tc.tile_pool(name="glob", bufs=1) as pp,
            tc.tile_pool(name="dram", bufs=1, space="DRAM") as dramp,
            tc.tile_pool(name="qcnp", bufs=2) as qcnp,
        ):
            krotT = pp.tile([ROPE, S], BF16, tag="krotT")
            krot_cs = [pp.tile([ROPE, CW], BF16, tag=f"krotc{i}",
                               name=f"krotc{i}")
                       for i in range(len(CHUNKS) - 1)]
            kn_sb = pp.tile([P, HL, S], BF16, tag="kn_sb")
            vaug = pp.tile([P, HL, TT, VHD + 16], BF16, tag="vaug")
            costok = pp.tile([P, TT, ROPE], BF16, tag="costok")
            sintok = pp.tile([P, TT, ROPE], BF16, tag="sintok")
            masks_sb = pp.tile([P, 896], BF16, tag="masks")
            cosA_sb = pp.tile([ROPE, 896], BF16, tag="cosA")
            sinA_sb = pp.tile([ROPE, 896], BF16, tag="sinA")
            ones_sb = pp.tile([P, P], BF16, tag="ones")
            ident_sb = pp.tile([P, P], BF16, tag="ident")
            eps_sb = pp.tile([P, 1], F32, tag="eps")

            gin = [dramp.tile([NFT, P, w // 4], BF16, tag=f"gin{i+1}",
                              name=f"gin{i+1}")
                   for i, (_, w) in enumerate(CHUNKS[1:])]
            gout_tiles = [dramp.tile([4, NFT, P, w // 4], BF16,
                                     tag=f"gout{i+1}", name=f"gout{i+1}")
                          for i, (_, w) in enumerate(CHUNKS[1:])]

            nc.vector.memset(eps_sb[:], EPS)
            nc.vector.memset(ones_sb[:], 1.0)
            nc.vector.memset(vaug[:, :, :, VHD], 1.0)
            bmasks.make_identity(nc, ident_sb[:])
            warm = pp.tile([P, 1], F32, tag="warm")
            nc.scalar.activation(warm[:], eps_sb[:], AF.Sqrt)

            # chunk-0 latents are produced locally by stage A
            qcn0 = qcnp.tile([P, KQ + KKV, CW], BF16, tag="qcn", name="qcn0")
            qnT0 = qcn0[:, 0:KQ]
            cnT0 = qcn0[:, KQ:KQ + KKV]

            stg_tiles = _emit_stage_a(
                nc, tc, pp, gin, qnT0, cnT0, krotT, cosA_sb,
                sinA_sb, ones_sb, eps_sb, hT_own, wqaT, wkvaT,
                cosA, sinA)
            # write-write deps keep these bulk table loads off the DMA
            # engines until the gather slices are staged
            last_stg = stg_tiles[0]
            nc.vector.tensor_copy(masks_sb[0:1, 0:1], last_stg[0:1, 0, 0:1])
            nc.vector.tensor_copy(costok[0:1, 0:1, 0:1], last_stg[0:1, 0, 0:1])
            nc.vector.tensor_copy(sintok[0:1, 0:1, 0:1], last_stg[0:1, 0, 0:1])
            nc.scalar.dma_start(masks_sb[:], masks[:])
            nc.scalar.dma_start(
                costok[:], cos_tm[:].rearrange("(t p) r -> p t r", p=P))
            nc.scalar.dma_start(
                sintok[:], sin_tm_s[:].rearrange("(t p) r -> p t r", p=P))

            # ====== Stages B + C + D, interleaved per 512-token chunk ======
            with (
                tc.tile_pool(name="wB", bufs=1) as wB,
                tc.tile_pool(name="qnopep", bufs=2) as qnopep,
                tc.tile_pool(name="qpep", bufs=2) as qpep,
                tc.tile_pool(name="attnp", bufs=2) as attnp,
                tc.tile_pool(name="ptp", bufs=28) as ptp,
                tc.tile_pool(name="obp", bufs=3) as obp,
                tc.tile_pool(name="scrB", bufs=6) as scrB,
                tc.tile_pool(name="psB", bufs=2, space="PSUM") as psB,
                tc.tile_pool(name="psVPE", bufs=2, space="PSUM") as psVPE,
                tc.tile_pool(name="psSO", bufs=3, space="PSUM") as psSO,
                tc.tile_pool(name="psT", bufs=1, space="PSUM") as psT,
            ):
                trslot = [0]
                trt = psT.tile([P, 8, P], BF16, tag="trps", name="trt")

                def tr_tile():
                    s = trslot[0] % 8
                    trslot[0] += 1
                    return trt[:, s]

                wqb_sb = wB.tile([P, KQ, HL * QHD], BF16, tag="wqb")
                wkvb_sb = wB.tile([P, KKV, HL * (NOPE + VHD)], BF16, tag="wkvb")
                wo_sb = wB.tile([P, HL, HIDDEN], BF16, tag="wo")
                nc.sync.dma_start(
                    wqb_sb[:], wqbT[:].rearrange("(k p) f -> p k f", p=P))
                nc.sync.dma_start(
                    wkvb_sb[:], wkvbT[:].rearrange("(k p) f -> p k f", p=P))
                nc.sync.dma_start(
                    wo_sb[:], woT[:].rearrange("(k p) f -> p k f", p=P))

                chunk_floor_ms = [0.0, 0.13, 0.20, 0.26, 0.29]

                def krot_src(kt):
                    for ci2, (t0, w2) in enumerate(CHUNKS):
                        if t0 // P <= kt < (t0 + w2) // P:
                            loc = kt - t0 // P
                            if ci2 == 0:
                                return krotT[:, kt * P:(kt + 1) * P]
                            return krot_cs[ci2 - 1][:, loc * P:(loc + 1) * P]
                    raise AssertionError(kt)

                for c, (c0, W) in enumerate(CHUNKS):
                    tc.tile_set_cur_wait(chunk_floor_ms[c])
                    NT = W // P          # query token tiles in this chunk
                    KT0 = c0 // P        # first key tile index of this chunk
                    W4 = W // 4          # per-core gather slice width
                    if c == 0:
                        qnT_c, cnT_c = qnT0, cnT0
                    else:
                        g = gout_tiles[c - 1]
                        qcn_c = qcnp.tile([P, KQ + KKV, CW], BF16, tag="qcn",
                                          name=f"qcn{c}")
                        qnT_c = qcn_c[:, 0:KQ]
                        cnT_c = qcn_c[:, KQ:KQ + KKV]
                        for r in range(4):
                            nc.gpsimd.dma_start(
                                qcn_c[:, KQ:KQ + KKV, r * W4:(r + 1) * W4],
                                g[r, KQ:KQ + KKV].rearrange("f p t -> p f t"))
                        for r in range(4):
                            nc.gpsimd.dma_start(
                                qcn_c[:, 0:KQ, r * W4:(r + 1) * W4],
                                g[r, 0:KQ].rearrange("f p t -> p f t"))
                        nc.gpsimd.dma_start(
                            krot_cs[c - 1][:, 0:W].rearrange(
                                "p (r t) -> p r t", r=4),
                            g[:, KQ + KKV, 0:ROPE].rearrange("r p t -> p r t"))

                    # ---- B: up-projections for this chunk ----
                    for h in range(HL):
                        ps = psB.tile([P, CW], F32, tag="bps")
                        for k in range(KKV):
                            nc.tensor.matmul(
                                ps[:, 0:W], wkvb_sb[:, k, h * P:(h + 1) * P],
                                cnT_c[:, k, 0:W], start=(k == 0),
                                stop=(k == KKV - 1))
                        nc.scalar.copy(kn_sb[:, h, c0:c0 + W], ps[:, 0:W])
                    for h in range(HL):
                        pv = psVPE.tile([P, 4, VHD], F32, tag="vpe")
                        for tt in range(NT):
                            for k in range(KKV):
                                nc.tensor.matmul(
                                    pv[:, tt], cnT_c[:, k, tt * P:(tt + 1) * P],
                                    wkvb_sb[:, k, HL * P + h * P:HL * P + (h + 1) * P],
                                    start=(k == 0), stop=(k == KKV - 1))
                        nc.vector.tensor_copy(
                            vaug[:, h, KT0:KT0 + NT, 0:VHD], pv[:, 0:NT])
                    qnope_c = qnopep.tile([P, HL, CW], BF16, tag="qnope",
                                          name=f"qnope{c}")
                    for h in range(HL):
                        ps = psB.tile([P, CW], F32, tag="bps")
                        for k in range(KQ):
                            nc.tensor.matmul(
                                ps[:, 0:W], wqb_sb[:, k, h * P:(h + 1) * P],
                                qnT_c[:, k, 0:W], start=(k == 0),
                                stop=(k == KQ - 1))
                        nc.scalar.copy(qnope_c[:, h, 0:W], ps[:, 0:W])
                    qpe_c = qpep.tile([ROPE, HL, CW], BF16, tag="qpe",
                                      name=f"qpe{c}")
                    pend_tr = []

                    def flush_tr():
                        for qr_, h_, tt_ in pend_tr:
                            tr = tr_tile()
                            nc.tensor.transpose(tr[0:ROPE, :], qr_[:, h_],
                                                ident_sb[:])
                            nc.vector.tensor_copy(
                                qpe_c[:, h_, tt_ * P:(tt_ + 1) * P],
                                tr[0:ROPE, :])
                        del pend_tr[:]

                    for tt in range(NT):
                        pspe_t = psVPE.tile([P, 4, VHD], F32, tag="vpe",
                                            name=f"pspe{c}_{tt}")
                        pspe = pspe_t[:, 0:2].rearrange("p a b -> p (a b)")
                        for k in range(KQ):
                            nc.tensor.matmul(
                                pspe[:], qnT_c[:, k, tt * P:(tt + 1) * P],
                                wqb_sb[:, k, HL * P:HL * P + HL * ROPE],
                                start=(k == 0), stop=(k == KQ - 1))
                        t_glob = KT0 + tt
                        qr = scrB.tile([P, HL, ROPE], BF16, tag="qr")
                        qtmp = scrB.tile([P, ROPE], BF16, tag="qtmp")
                        for h in range(HL):
                            hs = h * ROPE
                            nc.vector.tensor_mul(
                                qr[:, h], pspe[:, hs:hs + ROPE], costok[:, t_glob])
                            nc.vector.tensor_mul(
                                qtmp[:, 0:32], pspe[:, hs + 32:hs + ROPE],
                                sintok[:, t_glob, 0:32])
                            nc.vector.tensor_mul(
                                qtmp[:, 32:ROPE], pspe[:, hs:hs + 32],
                                sintok[:, t_glob, 32:ROPE])
                            nc.vector.tensor_add(qr[:, h], qr[:, h], qtmp[:])
                            tr = tr_tile()
                            nc.tensor.transpose(tr[0:ROPE, :], qr[:, h],
                                                ident_sb[:])
                            nc.vector.tensor_copy(
                                qpe_c[:, h, tt * P:(tt + 1) * P], tr[0:ROPE, :])

                    # ---- C: attention for this chunk's queries ----
                    attn_c = attnp.tile([P, HL, CW], BF16, tag="attn",
                                        name=f"attn{c}")
                    nkt = KT0 + NT

                    def emit_pv(h, pts, offs, NT=NT, KT0=KT0):
                        for qs in range(NT):
                            qt = KT0 + qs
                            po_t = psSO.tile([P, CW], F32, tag="sso",
                                             name=f"po{c}_{h}_{qs}")
                            po = po_t[:, 0:VHD + 1]
                            for kt in range(qt + 1):
                                o = qs * P - offs[kt]
                                nc.tensor.matmul(
                                    po[:], pts[kt][:, o:o + P],
                                    vaug[:, h, kt, 0:VHD + 1],
                                    start=(kt == 0), stop=(kt == qt))
                            rd = scrB.tile([P, 1], F32, tag="rd")
                            nc.vector.reciprocal(rd[:], po[:, VHD:VHD + 1])
                            at = scrB.tile([P, VHD], BF16, tag="at")
                            nc.vector.tensor_scalar_mul(at[:], po[:, 0:VHD], rd[:])
                            tr2 = tr_tile()
                            nc.tensor.transpose(tr2[:], at[:], ident_sb[:])
                            nc.vector.tensor_copy(
                                attn_c[:, h, qs * P:(qs + 1) * P], tr2[:])

                    prev = None
                    for h in range(HL):
                        pts = []
                        offs = []
                        for kt in range(nkt):
                            diag = (kt >= KT0)
                            off = (kt - KT0) * P if diag else 0
                            w = W - off
                            ss = psSO.tile([P, CW], F32, tag="sso")
                            nc.tensor.matmul(
                                ss[:, 0:w], kn_sb[:, h, kt * P:(kt + 1) * P],
                                qnope_c[:, h, off:W], start=True, stop=False)
                            nc.tensor.matmul(
                                ss[:, 0:w], krot_src(kt),
                                qpe_c[:, h, off:W], start=False, stop=True)
                            pt = ptp.tile([P, CW], BF16, tag="pt")
                            nc.scalar.activation(pt[:, 0:w], ss[:, 0:w], AF.Exp,
                                                 scale=SOFT_SCALE)
                            if diag:
                                nc.vector.tensor_mul(
                                    pt[:, 0:P], pt[:, 0:P],
                                    masks_sb[:, 384:384 + P])
                            pts.append(pt)
                            offs.append(off)
                            if kt == min(7, nkt - 1) and prev is not None:
                                emit_pv(*prev)
                                prev = None
                        if prev is not None:
                            emit_pv(*prev)
                        prev = (h, pts, offs)
                    emit_pv(*prev)

                    # ---- D: wo projection for this chunk ----
                    for og in range(4):
                        ob = obp.tile([P, 4, CW], BF16, tag="ob")
                        for oi in range(4):
                            ot = og * 4 + oi
                            ps = psB.tile([P, CW], F32, tag="bps")
                            for k in range(HL):
                                nc.tensor.matmul(
                                    ps[:, 0:W], wo_sb[:, k, ot * P:(ot + 1) * P],
                                    attn_c[:, k, 0:W], start=(k == 0),
                                    stop=(k == HL - 1))
                            if oi % 2 == 0:
                                nc.scalar.copy(ob[:, oi, 0:W], ps[:, 0:W])
                            else:
                                nc.vector.tensor_copy(ob[:, oi, 0:W], ps[:, 0:W])
                        nc.sync.dma_start(
                            outT[og * 4 * P:(og + 1) * 4 * P,
                                 c0:c0 + W].rearrange("(o p) t -> p o t", p=P),
                            ob[:, :, 0:W])

    nc.compile()
    return nc


_NC_CACHE = {}
_LAST_RES = None


def _get_nc(stages="ALL"):
    if "nc" not in _NC_CACHE:
        _NC_CACHE["nc"] = _build_nc()
    return _NC_CACHE["nc"]


def kernel(hidden_states, position_ids, wq_a, q_a_ln_w, wq_b, wkv_a, kv_a_ln_w,
           wkv_b, wo):
    hidden_states = np.asarray(hidden_states, dtype=np.float32)
    position_ids = np.asarray(position_ids)
    wq_a = np.asarray(wq_a, dtype=np.float32)
    wq_b = np.asarray(wq_b, dtype=np.float32)
    wkv_a = np.asarray(wkv_a, dtype=np.float32)
    wkv_b = np.asarray(wkv_b, dtype=np.float32)
    wo = np.asarray(wo, dtype=np.float32)
    # fold RMSNorm elementwise weights into the up-projections (exact)
    wq_b = wq_b * np.asarray(q_a_ln_w, dtype=np.float32)[None, :]
    wkv_b = wkv_b * np.asarray(kv_a_ln_w, dtype=np.float32)[None, :]
    assert hidden_states.shape == (B, S, HIDDEN)

    cos_t, sin_t = _yarn_cos_sin(S)

    # --- weight preprocessing (shared across cores in each batch group) ---
    wqbT_groups = []
    wkvbT_groups = []
    woT_groups = []
    for g in range(4):
        heads = range(4 * g, 4 * g + 4)
        rows = []
        for h in heads:
            rows.append(np.arange(h * QHD, h * QHD + NOPE))
        pe_rows = []
        for h in heads:
            pe_rows.append(h * QHD + NOPE + _PERM64)
        rows = np.concatenate(rows + pe_rows)
        wqbT_groups.append(_bf16(wq_b[rows].T))

        rows = []
        for h in heads:
            rows.append(np.arange(h * (NOPE + VHD), h * (NOPE + VHD) + NOPE))
        for h in heads:
            rows.append(np.arange(h * (NOPE + VHD) + NOPE, (h + 1) * (NOPE + VHD)))
        rows = np.concatenate(rows)
        wkvbT_groups.append(_bf16(wkv_b[rows].T))

        cols = np.concatenate([np.arange(h * VHD, (h + 1) * VHD) for h in heads])
        woT_groups.append(_bf16(wo[:, cols].T))

    wqaT = _bf16(wq_a.T)
    wkva_perm = wkv_a.copy()
    wkva_perm[KV_LORA:] = wkv_a[KV_LORA + _PERM64]
    wkvaT = _bf16(wkva_perm.T)

    x_idx = np.arange(896)[None, :]
    p_idx = np.arange(P)[:, None]
    masks = _bf16((x_idx >= 384 + p_idx).astype(np.float32))

    # --- per-core inputs ---
    in_maps = []
    for c in range(N_CORES):
        beta, g = c // 4, c % 4
        pos = position_ids[beta].astype(np.int64)
        cg = cos_t[pos]          # [S, 64]
        sg = sin_t[pos]
        sin_s = np.concatenate([-sg[:, :32], sg[:, 32:]], axis=1)

        own = np.concatenate(
            [np.arange(CHUNKS[0][1])] +
            [np.arange(t0 + g * (w // 4), t0 + (g + 1) * (w // 4))
             for t0, w in CHUNKS[1:]])
        hT_own = _bf16(hidden_states[beta].T[:, own])
        cosA = _bf16(cg[own].T)
        sg_own = sg[own]
        sinA = _bf16(np.concatenate([-sg_own[:, :32].T, sg_own[:, 32:].T], axis=0))

        in_maps.append({
            "hT_own": hT_own,
            "wqaT": wqaT,
            "wkvaT": wkvaT,
            "wqbT": wqbT_groups[g],
            "wkvbT": wkvbT_groups[g],
            "woT": woT_groups[g],
            "cosA": cosA,
            "sinA": sinA,
            "cos_tm": _bf16(cg),
            "sin_tm_s": _bf16(sin_s),
            "masks": masks,
        })

    nc = _get_nc()
    global _LAST_RES
    res = run_bass_kernel_spmd(nc, in_maps, core_ids=list(range(N_CORES)))
    _LAST_RES = res

    out = np.zeros((B, S, HIDDEN), dtype=np.float32)
    for c in range(N_CORES):
        out[c // 4] += res.results[c]["outT"].astype(np.float32).T
    return out


# revision 44
# speedup vs baseline: 1.0124x; 1.0124x over previous
"""DeepseekV3 MLA attention forward on 8 Trainium2 NeuronCores.

Sharding: core c -> batch c//4, head group c%4 (4 of 16 heads).

v2: the low-rank down-projections (stage A) are token-sharded across the
4 cores of each batch group instead of replicated. Each core computes the
down-projection + RMSNorm + k-rope for its own slice of token chunks 1+
(chunk 0 is computed in full by every core, which is cheaper than another
gather), then per-chunk AllGathers (replica groups [[0,1,2,3],[4,5,6,7]])
distribute the normalized latents. The gathers run on the collective cores
while the PE works through chunk 0, so they are fully hidden. Chunk widths
shrink toward the end of the sequence ([512,512,512,384,128]) so the
serial tail behind the last gather is short.

Stage A is feature-major (weights stationary, tokens moving): outputs land
directly in the [feature, token] layout stages B/C consume, eliminating all
DMA transposes. Per-token RMS norms are computed with an all-ones stationary
matmul (partition reduction, replicated across partitions) so scaling is a
plain elementwise multiply. The remaining transposes (q-rope heads, attn
output) run on the PE via identity-matmul transpose.

B (up-projections), C (attention), D (wo projection) are interleaved per
512-token chunk so the output projection and DMA overlap attention for the
next chunk. Matmuls in bf16 with f32 PSUM accumulation; exp without max
subtraction; multiplicative causal mask tiles; ones-augmented V gives the
softmax denominator for free. wo partial products are returned in bf16 and
summed on host.
"""

import math

import numpy as np
import ml_dtypes

import concourse.bass as bass
import concourse.tile as tile
import concourse.mybir as mybir
from concourse import bacc
from concourse import masks as bmasks
from concourse.bass_utils import run_bass_kernel_spmd

BF16 = mybir.dt.bfloat16
F32 = mybir.dt.float32
AF = mybir.ActivationFunctionType

# ---- model config (hardcoded to match the problem spec) ----
HIDDEN = 2048
N_HEADS = 16
Q_LORA = 1536
KV_LORA = 512
NOPE = 128
ROPE = 64
VHD = 128
QHD = NOPE + ROPE  # 192
BASE = 10000.0
SCALE = 40.0
ORIG_MAX = 4096
BETA_FAST = 32
BETA_SLOW = 1
EPS = 1e-6
B = 2
S = 2048

N_CORES = 8
HL = 4           # heads per core
P = 128
KH = HIDDEN // P     # 16
KQ = Q_LORA // P     # 12
KKV = KV_LORA // P   # 4
NFT = KQ + KKV + 1   # 17 down-proj feature tiles (12 q, 4 kv, 1 rope)
NC_CHUNK = 4         # (legacy) uniform chunk count
CW = 512             # max chunk width / psum tile width
# token chunks: (start, width). A smaller tail chunk shortens the serial
# dependency chain behind the last all-gather.
CHUNKS = [(0, 512), (512, 512), (1024, 512), (1536, 384), (1920, 128)]
# per-core gather slice widths for chunks 1.. (width // 4)
SLICES = [(512 + sum(w for _, w in CHUNKS[1:1 + i]) // 4, w // 4)
          for i, (_, w) in enumerate(CHUNKS[1:])]
# hT_own column bases for the gather slices
_sl_bases = []
_acc = 512
for _, w in CHUNKS[1:]:
    _sl_bases.append((_acc, w // 4))
    _acc += w // 4

_m = 0.1 * math.log(SCALE) + 1.0
SOFT_SCALE = (QHD ** -0.5) * _m * _m

REPLICA_GROUPS = [[0, 1, 2, 3], [4, 5, 6, 7]]


def _yarn_cos_sin(seq_len):
    dim = ROPE
    ar = np.arange(0, dim, 2, dtype=np.float32)
    freq_extra = 1.0 / BASE ** (ar / dim)
    freq_inter = 1.0 / (SCALE * BASE ** (ar / dim))
    low = math.floor(dim * math.log(ORIG_MAX / (BETA_FAST * 2 * math.pi)) / (2 * math.log(BASE)))
    high = math.ceil(dim * math.log(ORIG_MAX / (BETA_SLOW * 2 * math.pi)) / (2 * math.log(BASE)))
    low, high = max(low, 0), min(high, dim - 1)
    denom = (high - low) if high != low else 0.001
    ramp = np.clip((np.arange(dim // 2, dtype=np.float32) - low) / denom, 0.0, 1.0)
    inv_freq_mask = 1.0 - ramp
    inv_freq = freq_inter * (1.0 - inv_freq_mask) + freq_extra * inv_freq_mask
    t = np.arange(seq_len, dtype=np.float32)
    freqs = np.outer(t, inv_freq)
    emb = np.concatenate([freqs, freqs], axis=-1)
    # mscale ratio is 1.0 for this config
    return np.cos(emb).astype(np.float32), np.sin(emb).astype(np.float32)


_PERM64 = np.concatenate([np.arange(0, 64, 2), np.arange(1, 64, 2)])


def _bf16(x):
    return np.ascontiguousarray(x.astype(ml_dtypes.bfloat16))


def _emit_stage_a(nc, tc, pp, gin, qnT0, cnT0, krotT, cosA_sb, sinA_sb,
                  ones_sb, eps_sb, hT_own, wqaT, wkvaT, cosA, sinA):
    """Down-projections, RMS norms, k-rope for this core's tokens.

    7 uniform 128-token stages: own slices of chunks 1..3 (staged to DRAM
    and all-gathered), then the 4 quarters of chunk 0 (kept local)."""
    stg_tiles = []
    with (
        tc.tile_pool(name="wA", bufs=1) as wA,
        tc.tile_pool(name="htp", bufs=3) as htp,
        tc.tile_pool(name="stgp", bufs=2) as stgp,
        tc.tile_pool(name="sqp", bufs=18) as sqp,
        tc.tile_pool(name="scrA", bufs=6) as scrA,
        tc.tile_pool(name="psq", bufs=2, space="PSUM") as psq,
        tc.tile_pool(name="pskv", bufs=1, space="PSUM") as pskv,
        tc.tile_pool(name="psnr", bufs=1, space="PSUM") as psnr,
    ):
        wqa_sb = wA.tile([P, KH, Q_LORA], BF16, tag="wqa")
        wkva_sb = wA.tile([P, KH, KV_LORA + ROPE], BF16, tag="wkva")

        # (hT_own column base, width, gather slice index or None, quarter)
        stages = [(b, w, i + 1, None) for i, (b, w) in enumerate(_sl_bases)]
        stages += [(m * P, P, None, m) for m in range(4)]
        nsl = len(_sl_bases)
        # prefetch the gather slices' tokens, then the weights (single
        # batched DMAs: per-transfer overhead dominates small copies)
        ht_pre = {}
        for i, (colbase, w, _, _) in enumerate(stages[:nsl]):
            ht_pre[colbase] = htp.tile([P, KH, P], BF16, tag="ht",
                                       name=f"htp{colbase}")
            nc.sync.dma_start(
                ht_pre[colbase][:, :, 0:w],
                hT_own[:, colbase:colbase + w].rearrange("(k p) t -> p k t", p=P))
            if i == 0:
                for vg in range(2):
                    nc.sync.dma_start(
                        wkva_sb[:, :, vg * 288:(vg + 1) * 288],
                        wkvaT[:, vg * 288:(vg + 1) * 288].rearrange(
                            "(k p) f -> p k f", p=P))
        nc.vector.tensor_copy(wqa_sb[0:1, 0:1, 0:1], wkva_sb[0:1, 0:1, 0:1])
        nc.vector.tensor_copy(wqa_sb[0:1, 0:1, 1:2], ht_pre[_sl_bases[2][0]][0:1, 0:1, 0:1])
        for wg in range(3):
            nc.sync.dma_start(
                wqa_sb[:, :, wg * CW:(wg + 1) * CW],
                wqaT[:, wg * CW:(wg + 1) * CW].rearrange(
                    "(k p) f -> p k f", p=P))
        nc.sync.dma_start(cosA_sb[:], cosA[:])
        nc.sync.dma_start(sinA_sb[:], sinA[:])
        for colbase, wd, gslice, quarter in stages:
            if colbase in ht_pre:
                ht = ht_pre[colbase]
            else:
                ht = htp.tile([P, KH, P], BF16, tag="ht",
                              name=f"ht{colbase}")
                nc.sync.dma_start(
                    ht[:, :, 0:wd], hT_own[:, colbase:colbase + wd].rearrange(
                        "(k p) t -> p k t", p=P))
            ps_q = psq.tile([P, KQ, P], F32, tag="psq")
            ps_kv = pskv.tile([P, KKV, P], F32, tag="kvps")
            ps_nr = psnr.tile([P, 3, P], F32, tag="nrps")
            for f2 in range(KKV):
                for k in range(KH):
                    nc.tensor.matmul(
                        ps_kv[:, f2, 0:wd], wkva_sb[:, k, f2 * P:(f2 + 1) * P],
                        ht[:, k, 0:wd], start=(k == 0), stop=(k == KH - 1))
            for k in range(KH):
                nc.tensor.matmul(
                    ps_nr[0:ROPE, 2, 0:wd],
                    wkva_sb[:, k, KV_LORA:KV_LORA + ROPE], ht[:, k, 0:wd],
                    start=(k == 0), stop=(k == KH - 1))
            # kv norm drains early (fills the PE idle while wqa streams in)
            sg = scrA.tile([P, 2, P], F32, tag="sg")
            for f2 in range(KKV):
                sq = sqp.tile([P, P], BF16, tag="sq")
                nc.scalar.activation(sq[:, 0:wd], ps_kv[:, f2, 0:wd], AF.Square)
                nc.tensor.matmul(ps_nr[:, 1, 0:wd], ones_sb[:],
                                 sq[:, 0:wd],
                                 start=(f2 == 0), stop=(f2 == KKV - 1))
            nc.scalar.activation(sg[:, 1, 0:wd], ps_nr[:, 1, 0:wd],
                                 AF.Sqrt, scale=1.0 / KV_LORA, bias=eps_sb[:])
            rkv = scrA.tile([P, P], F32, tag="rkv")
            nc.vector.reciprocal(rkv[:, 0:wd], sg[:, 1, 0:wd])

            for ft in range(KQ):
                for k in range(KH):
                    nc.tensor.matmul(
                        ps_q[:, ft, 0:wd], wqa_sb[:, k, ft * P:(ft + 1) * P],
                        ht[:, k, 0:wd], start=(k == 0), stop=(k == KH - 1))
            for ft in range(KQ):
                sq = sqp.tile([P, P], BF16, tag="sq")
                nc.scalar.activation(sq[:, 0:wd], ps_q[:, ft, 0:wd], AF.Square)
                nc.tensor.matmul(ps_nr[:, 0, 0:wd], ones_sb[:],
                                 sq[:, 0:wd],
                                 start=(ft == 0), stop=(ft == KQ - 1))
            nc.scalar.activation(sg[:, 0, 0:wd], ps_nr[:, 0, 0:wd],
                                 AF.Sqrt, scale=1.0 / Q_LORA, bias=eps_sb[:])
            rq = scrA.tile([P, P], F32, tag="rq")
            nc.vector.reciprocal(rq[:, 0:wd], sg[:, 0, 0:wd])

            if gslice is not None:
                stg = stgp.tile([P, NFT, P], BF16, tag="stg")
                stg_tiles.append(stg)
                dq = [stg[:, ft, 0:wd] for ft in range(KQ)]
                dc = [stg[:, KQ + f2, 0:wd] for f2 in range(KKV)]
                dkr = stg[0:ROPE, KQ + KKV, 0:wd]
                nc.vector.memset(stg[ROPE:P, KQ + KKV, 0:wd], 0.0)
            else:
                m0 = quarter * P
                dq = [qnT0[:, ft, m0:m0 + P] for ft in range(KQ)]
                dc = [cnT0[:, f2, m0:m0 + P] for f2 in range(KKV)]
                dkr = krotT[:, m0:m0 + P]
            for ft in range(KQ):
                nc.vector.tensor_mul(dq[ft], ps_q[:, ft, 0:wd], rq[:, 0:wd])
            for f2 in range(KKV):
                nc.vector.tensor_mul(dc[f2], ps_kv[:, f2, 0:wd], rkv[:, 0:wd])
            # decoupled rope on the shared key (feature-major, true scale)
            ca = cosA_sb[:, colbase:colbase + wd]
            sa = sinA_sb[:, colbase:colbase + wd]
            tmp = scrA.tile([ROPE, P], BF16, tag="tmpr")
            nc.vector.tensor_mul(dkr, ps_nr[0:ROPE, 2, 0:wd], ca)
            nc.vector.tensor_mul(tmp[0:32, 0:wd], ps_nr[32:ROPE, 2, 0:wd],
                                 sa[0:32])
            nc.vector.tensor_mul(tmp[32:ROPE, 0:wd], ps_nr[0:32, 2, 0:wd],
                                 sa[32:ROPE])
            nc.vector.tensor_add(dkr, dkr, tmp[0:ROPE, 0:wd])

            if gslice is not None:
                nc.sync.dma_start(
                    gin[gslice - 1][:].rearrange("f p t -> p f t"),
                    stg[:, :, 0:wd])
                nc.gpsimd.collective_compute(
                    "AllGather",
                    mybir.AluOpType.bypass,
                    replica_groups=REPLICA_GROUPS,
                    ins=[gin[gslice - 1].opt()],
                    outs=[gout_tiles[gslice - 1].opt()],
                )
    return stg_tiles


def _build_nc():
    nc = bacc.Bacc("TRN2", target_bir_lowering=False, debug=False,
                   num_devices=N_CORES)

    hT_own = nc.declare_dram_parameter("hT_own", [HIDDEN, 896], BF16, isOutput=False)
    wqaT = nc.declare_dram_parameter("wqaT", [HIDDEN, Q_LORA], BF16, isOutput=False)
    wkvaT = nc.declare_dram_parameter("wkvaT", [HIDDEN, KV_LORA + ROPE], BF16, isOutput=False)
    wqbT = nc.declare_dram_parameter("wqbT", [Q_LORA, HL * QHD], BF16, isOutput=False)
    wkvbT = nc.declare_dram_parameter("wkvbT", [KV_LORA, HL * (NOPE + VHD)], BF16, isOutput=False)
    woT = nc.declare_dram_parameter("woT", [HL * VHD, HIDDEN], BF16, isOutput=False)
    cosA = nc.declare_dram_parameter("cosA", [ROPE, 896], BF16, isOutput=False)
    sinA = nc.declare_dram_parameter("sinA", [ROPE, 896], BF16, isOutput=False)
    cos_tm = nc.declare_dram_parameter("cos_tm", [S, ROPE], BF16, isOutput=False)
    sin_tm_s = nc.declare_dram_parameter("sin_tm_s", [S, ROPE], BF16, isOutput=False)
    masks = nc.declare_dram_parameter("masks", [P, 896], BF16, isOutput=False)
    outT = nc.declare_dram_parameter("outT", [HIDDEN, S], BF16, isOutput=True)

    TT = S // P

    global gout_tiles
    with tile.TileContext(nc) as tc:
        with (
            tc.tile_pool(name="glob", bufs=1) as pp,
            tc.tile_pool(name="dram", bufs=1, space="DRAM") as dramp,
            tc.tile_pool(name="qcnp", bufs=2) as qcnp,
        ):
            krotT = pp.tile([ROPE, S], BF16, tag="krotT")
            krot_cs = [pp.tile([ROPE, CW], BF16, tag=f"krotc{i}",
                               name=f"krotc{i}")
                       for i in range(len(CHUNKS) - 1)]
            kn_sb = pp.tile([P, HL, S], BF16, tag="kn_sb")
            vaug = pp.tile([P, HL, TT, VHD + 16], BF16, tag="vaug")
            costok = pp.tile([P, TT, ROPE], BF16, tag="costok")
            sintok = pp.tile([P, TT, ROPE], BF16, tag="sintok")
            masks_sb = pp.tile([P, 896], BF16, tag="masks")
            cosA_sb = pp.tile([ROPE, 896], BF16, tag="cosA")
            sinA_sb = pp.tile([ROPE, 896], BF16, tag="sinA")
            ones_sb = pp.tile([P, P], BF16, tag="ones")
            ident_sb = pp.tile([P, P], BF16, tag="ident")
            eps_sb = pp.tile([P, 1], F32, tag="eps")

            gin = [dramp.tile([NFT, P, w // 4], BF16, tag=f"gin{i+1}",
                              name=f"gin{i+1}")
                   for i, (_, w) in enumerate(CHUNKS[1:])]
            gout_tiles = [dramp.tile([4, NFT, P, w // 4], BF16,
                                     tag=f"gout{i+1}", name=f"gout{i+1}")
                          for i, (_, w) in enumerate(CHUNKS[1:])]

            nc.vector.memset(eps_sb[:], EPS)
            nc.vector.memset(ones_sb[:], 1.0)
            nc.vector.memset(vaug[:, :, :, VHD], 1.0)
            bmasks.make_identity(nc, ident_sb[:])
            warm = pp.tile([P, 1], F32, tag="warm")
            nc.scalar.activation(warm[:], eps_sb[:], AF.Sqrt)

            # chunk-0 latents are produced locally by stage A
            qcn0 = qcnp.tile([P, KQ + KKV, CW], BF16, tag="qcn", name="qcn0")
            qnT0 = qcn0[:, 0:KQ]
            cnT0 = qcn0[:, KQ:KQ + KKV]

            stg_tiles = _emit_stage_a(
                nc, tc, pp, gin, qnT0, cnT0, krotT, cosA_sb,
                sinA_sb, ones_sb, eps_sb, hT_own, wqaT, wkvaT,
                cosA, sinA)
            # write-write deps keep these bulk table loads off the DMA
            # engines until the gather slices are staged
            last_stg = stg_tiles[0]
            nc.vector.tensor_copy(masks_sb[0:1, 0:1], last_stg[0:1, 0, 0:1])
            nc.vector.tensor_copy(costok[0:1, 0:1, 0:1], last_stg[0:1, 0, 0:1])
            nc.vector.tensor_copy(sintok[0:1, 0:1, 0:1], last_stg[0:1, 0, 0:1])
            nc.scalar.dma_start(masks_sb[:], masks[:])
            nc.scalar.dma_start(
                costok[:], cos_tm[:].rearrange("(t p) r -> p t r", p=P))
            nc.scalar.dma_start(
                sintok[:], sin_tm_s[:].rearrange("(t p) r -> p t r", p=P))

            # ====== Stages B + C + D, interleaved per 512-token chunk ======
            with (
                tc.tile_pool(name="wB", bufs=1) as wB,
                tc.tile_pool(name="qnopep", bufs=2) as qnopep,
                tc.tile_pool(name="qpep", bufs=2) as qpep,
                tc.tile_pool(name="attnp", bufs=2) as attnp,
                tc.tile_pool(name="ptp", bufs=28) as ptp,
                tc.tile_pool(name="obp", bufs=3) as obp,
                tc.tile_pool(name="scrB", bufs=6) as scrB,
                tc.tile_pool(name="psVPE", bufs=2, space="PSUM") as psVPE,
                tc.tile_pool(name="psSO", bufs=5, space="PSUM") as psSO,
                tc.tile_pool(name="psT", bufs=1, space="PSUM") as psT,
            ):
                trslot = [0]
                trt = psT.tile([P, 8, P], BF16, tag="trps", name="trt")

                def tr_tile():
                    s = trslot[0] % 8
                    trslot[0] += 1
                    return trt[:, s]

                wqb_sb = wB.tile([P, KQ, HL * QHD], BF16, tag="wqb")
                wkvb_sb = wB.tile([P, KKV, HL * (NOPE + VHD)], BF16, tag="wkvb")
                wo_sb = wB.tile([P, HL, HIDDEN], BF16, tag="wo")
                nc.sync.dma_start(
                    wqb_sb[:], wqbT[:].rearrange("(k p) f -> p k f", p=P))
                nc.sync.dma_start(
                    wkvb_sb[:], wkvbT[:].rearrange("(k p) f -> p k f", p=P))
                nc.sync.dma_start(
                    wo_sb[:], woT[:].rearrange("(k p) f -> p k f", p=P))

                chunk_floor_ms = [0.0, 0.13, 0.20, 0.26, 0.29]

                def krot_src(kt):
                    for ci2, (t0, w2) in enumerate(CHUNKS):
                        if t0 // P <= kt < (t0 + w2) // P:
                            loc = kt - t0 // P
                            if ci2 == 0:
                                return krotT[:, kt * P:(kt + 1) * P]
                            return krot_cs[ci2 - 1][:, loc * P:(loc + 1) * P]
                    raise AssertionError(kt)

                for c, (c0, W) in enumerate(CHUNKS):
                    tc.tile_set_cur_wait(chunk_floor_ms[c])
                    NT = W // P          # query token tiles in this chunk
                    KT0 = c0 // P        # first key tile index of this chunk
                    W4 = W // 4          # per-core gather slice width
                    if c == 0:
                        qnT_c, cnT_c = qnT0, cnT0
                    else:
                        g = gout_tiles[c - 1]
                        qcn_c = qcnp.tile([P, KQ + KKV, CW], BF16, tag="qcn",
                                          name=f"qcn{c}")
                        qnT_c = qcn_c[:, 0:KQ]
                        cnT_c = qcn_c[:, KQ:KQ + KKV]
                        for r in range(4):
                            nc.gpsimd.dma_start(
                                qcn_c[:, KQ:KQ + KKV, r * W4:(r + 1) * W4],
                                g[r, KQ:KQ + KKV].rearrange("f p t -> p f t"))
                        for r in range(4):
                            nc.gpsimd.dma_start(
                                qcn_c[:, 0:KQ, r * W4:(r + 1) * W4],
                                g[r, 0:KQ].rearrange("f p t -> p f t"))
                        nc.gpsimd.dma_start(
                            krot_cs[c - 1][:, 0:W].rearrange(
                                "p (r t) -> p r t", r=4),
                            g[:, KQ + KKV, 0:ROPE].rearrange("r p t -> p r t"))

                    # ---- B: up-projections for this chunk ----
                    for h in range(HL):
                        ps = psSO.tile([P, CW], F32, tag="sso",
                                       name=f"knps{c}_{h}")
                        for k in range(KKV):
                            nc.tensor.matmul(
                                ps[:, 0:W], wkvb_sb[:, k, h * P:(h + 1) * P],
                                cnT_c[:, k, 0:W], start=(k == 0),
                                stop=(k == KKV - 1))
                        nc.scalar.copy(kn_sb[:, h, c0:c0 + W], ps[:, 0:W])
                    for h in range(HL):
                        pv = psVPE.tile([P, 4, VHD], F32, tag="vpe")
                        for tt in range(NT):
                            for k in range(KKV):
                                nc.tensor.matmul(
                                    pv[:, tt], cnT_c[:, k, tt * P:(tt + 1) * P],
                                    wkvb_sb[:, k, HL * P + h * P:HL * P + (h + 1) * P],
                                    start=(k == 0), stop=(k == KKV - 1))
                        nc.vector.tensor_copy(
                            vaug[:, h, KT0:KT0 + NT, 0:VHD], pv[:, 0:NT])
                    qnope_c = qnopep.tile([P, HL, CW], BF16, tag="qnope",
                                          name=f"qnope{c}")
                    for h in range(HL):
                        ps = psSO.tile([P, CW], F32, tag="sso",
                                       name=f"qnps{c}_{h}")
                        for k in range(KQ):
                            nc.tensor.matmul(
                                ps[:, 0:W], wqb_sb[:, k, h * P:(h + 1) * P],
                                qnT_c[:, k, 0:W], start=(k == 0),
                                stop=(k == KQ - 1))
                        nc.scalar.copy(qnope_c[:, h, 0:W], ps[:, 0:W])
                    qpe_c = qpep.tile([ROPE, HL, CW], BF16, tag="qpe",
                                      name=f"qpe{c}")
                    pend_tr = []

                    def flush_tr():
                        for qr_, h_, tt_ in pend_tr:
                            tr = tr_tile()
                            nc.tensor.transpose(tr[0:ROPE, :], qr_[:, h_],
                                                ident_sb[:])
                            nc.vector.tensor_copy(
                                qpe_c[:, h_, tt_ * P:(tt_ + 1) * P],
                                tr[0:ROPE, :])
                        del pend_tr[:]

                    for tt in range(NT):
                        pspe_t = psVPE.tile([P, 4, VHD], F32, tag="vpe",
                                            name=f"pspe{c}_{tt}")
                        pspe = pspe_t[:, 0:2].rearrange("p a b -> p (a b)")
                        for k in range(KQ):
                            nc.tensor.matmul(
                                pspe[:], qnT_c[:, k, tt * P:(tt + 1) * P],
                                wqb_sb[:, k, HL * P:HL * P + HL * ROPE],
                                start=(k == 0), stop=(k == KQ - 1))
                        t_glob = KT0 + tt
                        qr = scrB.tile([P, HL, ROPE], BF16, tag="qr")
                        qtmp = scrB.tile([P, ROPE], BF16, tag="qtmp")
                        for h in range(HL):
                            hs = h * ROPE
                            nc.vector.tensor_mul(
                                qr[:, h], pspe[:, hs:hs + ROPE], costok[:, t_glob])
                            nc.vector.tensor_mul(
                                qtmp[:, 0:32], pspe[:, hs + 32:hs + ROPE],
                                sintok[:, t_glob, 0:32])
                            nc.vector.tensor_mul(
                                qtmp[:, 32:ROPE], pspe[:, hs:hs + 32],
                                sintok[:, t_glob, 32:ROPE])
                            nc.vector.tensor_add(qr[:, h], qr[:, h], qtmp[:])
                            tr = tr_tile()
                            nc.tensor.transpose(tr[0:ROPE, :], qr[:, h],
                                                ident_sb[:])
                            nc.vector.tensor_copy(
                                qpe_c[:, h, tt * P:(tt + 1) * P], tr[0:ROPE, :])

                    # ---- C: attention for this chunk's queries ----
                    attn_c = attnp.tile([P, HL, CW], BF16, tag="attn",
                                        name=f"attn{c}")
                    nkt = KT0 + NT

                    def emit_pv(h, pts, offs, NT=NT, KT0=KT0):
                        for qs in range(NT):
                            qt = KT0 + qs
                            po_t = psSO.tile([P, CW], F32, tag="sso",
                                             name=f"po{c}_{h}_{qs}")
                            po = po_t[:, 0:VHD + 1]
                            for kt in range(qt + 1):
                                o = qs * P - offs[kt]
                                nc.tensor.matmul(
                                    po[:], pts[kt][:, o:o + P],
                                    vaug[:, h, kt, 0:VHD + 1],
                                    start=(kt == 0), stop=(kt == qt))
                            rd = scrB.tile([P, 1], F32, tag="rd")
                            nc.vector.reciprocal(rd[:], po[:, VHD:VHD + 1])
                            at = scrB.tile([P, VHD], BF16, tag="at")
                            nc.vector.tensor_scalar_mul(at[:], po[:, 0:VHD], rd[:])
                            tr2 = tr_tile()
                            nc.tensor.transpose(tr2[:], at[:], ident_sb[:])
                            nc.vector.tensor_copy(
                                attn_c[:, h, qs * P:(qs + 1) * P], tr2[:])

                    prev = None
                    for h in range(HL):
                        pts = []
                        offs = []
                        for kt in range(nkt):
                            diag = (kt >= KT0)
                            off = (kt - KT0) * P if diag else 0
                            w = W - off
                            ss = psSO.tile([P, CW], F32, tag="sso")
                            nc.tensor.matmul(
                                ss[:, 0:w], kn_sb[:, h, kt * P:(kt + 1) * P],
                                qnope_c[:, h, off:W], start=True, stop=False)
                            nc.tensor.matmul(
                                ss[:, 0:w], krot_src(kt),
                                qpe_c[:, h, off:W], start=False, stop=True)
                            pt = ptp.tile([P, CW], BF16, tag="pt")
                            nc.scalar.activation(pt[:, 0:w], ss[:, 0:w], AF.Exp,
                                                 scale=SOFT_SCALE)
                            if diag:
                                nc.vector.tensor_mul(
                                    pt[:, 0:P], pt[:, 0:P],
                                    masks_sb[:, 384:384 + P])
                            pts.append(pt)
                            offs.append(off)
                            if kt == min(7, nkt - 1) and prev is not None:
                                emit_pv(*prev)
                                prev = None
                        if prev is not None:
                            emit_pv(*prev)
                        prev = (h, pts, offs)
                    emit_pv(*prev)

                    # ---- D: wo projection for this chunk ----
                    for og in range(4):
                        ob = obp.tile([P, 4, CW], BF16, tag="ob")
                        for oi in range(4):
                            ot = og * 4 + oi
                            ps = psSO.tile([P, CW], F32, tag="sso",
                                           name=f"dps{c}_{og}_{oi}")
                            for k in range(HL):
                                nc.tensor.matmul(
                                    ps[:, 0:W], wo_sb[:, k, ot * P:(ot + 1) * P],
                                    attn_c[:, k, 0:W], start=(k == 0),
                                    stop=(k == HL - 1))
                            if oi % 2 == 0:
                                nc.scalar.copy(ob[:, oi, 0:W], ps[:, 0:W])
                            else:
                                nc.vector.tensor_copy(ob[:, oi, 0:W], ps[:, 0:W])
                        nc.sync.dma_start(
                            outT[og * 4 * P:(og + 1) * 4 * P,
                                 c0:c0 + W].rearrange("(o p) t -> p o t", p=P),
                            ob[:, :, 0:W])

    nc.compile()
    return nc


_NC_CACHE = {}
_LAST_RES = None


def _get_nc(stages="ALL"):
    if "nc" not in _NC_CACHE:
        _NC_CACHE["nc"] = _build_nc()
    return _NC_CACHE["nc"]


def kernel(hidden_states, position_ids, wq_a, q_a_ln_w, wq_b, wkv_a, kv_a_ln_w,
           wkv_b, wo):
    hidden_states = np.asarray(hidden_states, dtype=np.float32)
    position_ids = np.asarray(position_ids)
    wq_a = np.asarray(wq_a, dtype=np.float32)
    wq_b = np.asarray(wq_b, dtype=np.float32)
    wkv_a = np.asarray(wkv_a, dtype=np.float32)
    wkv_b = np.asarray(wkv_b, dtype=np.float32)
    wo = np.asarray(wo, dtype=np.float32)
    # fold RMSNorm elementwise weights into the up-projections (exact)
    wq_b = wq_b * np.asarray(q_a_ln_w, dtype=np.float32)[None, :]
    wkv_b = wkv_b * np.asarray(kv_a_ln_w, dtype=np.float32)[None, :]
    assert hidden_states.shape == (B, S, HIDDEN)

    cos_t, sin_t = _yarn_cos_sin(S)

    # --- weight preprocessing (shared across cores in each batch group) ---
    wqbT_groups = []
    wkvbT_groups = []
    woT_groups = []
    for g in range(4):
        heads = range(4 * g, 4 * g + 4)
        rows = []
        for h in heads:
            rows.append(np.arange(h * QHD, h * QHD + NOPE))
        pe_rows = []
        for h in heads:
            pe_rows.append(h * QHD + NOPE + _PERM64)
        rows = np.concatenate(rows + pe_rows)
        wqbT_groups.append(_bf16(wq_b[rows].T))

        rows = []
        for h in heads:
            rows.append(np.arange(h * (NOPE + VHD), h * (NOPE + VHD) + NOPE))
        for h in heads:
            rows.append(np.arange(h * (NOPE + VHD) + NOPE, (h + 1) * (NOPE + VHD)))
        rows = np.concatenate(rows)
        wkvbT_groups.append(_bf16(wkv_b[rows].T))

        cols = np.concatenate([np.arange(h * VHD, (h + 1) * VHD) for h in heads])
        woT_groups.append(_bf16(wo[:, cols].T))

    wqaT = _bf16(wq_a.T)
    wkva_perm = wkv_a.copy()
    wkva_perm[KV_LORA:] = wkv_a[KV_LORA + _PERM64]
    wkvaT = _bf16(wkva_perm.T)

    x_idx = np.arange(896)[None, :]
    p_idx = np.arange(P)[:, None]
    masks = _bf16((x_idx >= 384 + p_idx).astype(np.float32))

    # --- per-core inputs ---
    in_maps = []
    for c in range(N_CORES):
        beta, g = c // 4, c % 4
        pos = position_ids[beta].astype(np.int64)
        cg = cos_t[pos]          # [S, 64]
        sg = sin_t[pos]
        sin_s = np.concatenate([-sg[:, :32], sg[:, 32:]], axis=1)

        own = np.concatenate(
            [np.arange(CHUNKS[0][1])] +
            [np.arange(t0 + g * (w // 4), t0 + (g + 1) * (w // 4))
             for t0, w in CHUNKS[1:]])
        hT_own = _bf16(hidden_states[beta].T[:, own])
        cosA = _bf16(cg[own].T)
        sg_own = sg[own]
        sinA = _bf16(np.concatenate([-sg_own[:, :32].T, sg_own[:, 32:].T], axis=0))

        in_maps.append({
            "hT_own": hT_own,
            "wqaT": wqaT,
            "wkvaT": wkvaT,
            "wqbT": wqbT_groups[g],
            "wkvbT": wkvbT_groups[g],
            "woT": woT_groups[g],
            "cosA": cosA,
            "sinA": sinA,
            "cos_tm": _bf16(cg),
            "sin_tm_s": _bf16(sin_s),
            "masks": masks,
        })

    nc = _get_nc()
    global _LAST_RES
    res = run_bass_kernel_spmd(nc, in_maps, core_ids=list(range(N_CORES)))
    _LAST_RES = res

    out = np.zeros((B, S, HIDDEN), dtype=np.float32)
    for c in range(N_CORES):
        out[c // 4] += res.results[c]["outT"].astype(np.float32).T
    return out
